# revision 5
# baseline (speedup 1.0000x reference)
"""Bahdanau attention kernel for Trainium2 (8 NeuronCores, batch-sharded).

Problem (hardcoded):
  encoder_outputs [32, 2048, 1024] f32, decoder_hidden [32, 1024] f32,
  We [1024, 512], be [512], Wd [1024, 512], bd [512], v_w [512], v_b [] f32.
  Returns (context [32, 1024] f32, attn [32, 2048] f32).

Sharding: data-parallel over batch, 4 batches per core, weights replicated.

Per-core dataflow (batch-pipelined by the Tile scheduler):
  1. SWDGE cast-load enc f32->bf16 into native [128s, 1024e] tiles (kept for
     pass 2), xbar-DMA-transpose 128x128 blocks into encT [128e, s] layout.
  2. PE bf16 matmuls (lhsT=We block, rhs=encT) accumulate enc_e^T [a, s] in
     PSUM over the 8 e-blocks; ACT applies tanh with per-partition bias
     (dec_e + be + bd)[a]; PE contracts with v_w -> scores [1, S].
  3. Row softmax: DVE max, ACT exp (fused sum via accum_out), DVE scale.
  4. One SBUF->SBUF DMA sprays attn into column layout attnT [128s, 16st].
  5. PE pass-2 matmuls (lhsT=attnT col, rhs=native bf16 enc) accumulate
     context [1, 1024] over the 16 s-tiles.
"""

import numpy as np

import concourse.bass as bass
import concourse.mybir as mybir
import concourse.tile as tile
from concourse import bacc
from concourse.bass_utils import run_bass_kernel_spmd
from concourse.masks import make_identity

F32 = mybir.dt.float32
BF16 = mybir.dt.bfloat16

B, S, E, D, A = 32, 2048, 1024, 1024, 512
NCORES = 8
BPC = B // NCORES          # batches per core = 4
NST = S // 128             # s-tiles per batch = 16
NCH = 4                    # chunks per batch (4 s-tiles each)
STPC = NST // NCH          # s-tiles per chunk = 4
NEB = E // 128             # e blocks = 8
NAT = A // 128             # a tiles = 4
ATYPE = mybir.ActivationFunctionType


def build_kernel(tc, enc, dh, We, be, Wd, bd, vw, ctx_out, attn_out):
    nc = tc.nc

    with (
        tc.tile_pool(name="const", bufs=1) as const_pool,
        tc.tile_pool(name="enc_nat", bufs=34) as nat_pool,
        tc.tile_pool(name="encT", bufs=3) as encT_pool,
        tc.tile_pool(name="energy", bufs=6) as energy_pool,
        tc.tile_pool(name="small", bufs=2) as small_pool,
        tc.tile_pool(name="ps_main", bufs=4, space="PSUM") as ps_main,
        tc.tile_pool(name="ps_small", bufs=2, space="PSUM") as ps_small,
        tc.tile_pool(name="dram", bufs=2, space="DRAM") as dram_pool,
    ):
        # ---- replicated weights (a = at*128 + p layout for the a axis) ----
        We_bf = const_pool.tile([128, NEB, A], BF16)
        nc.gpsimd.dma_start(out=We_bf, in_=We.rearrange("(t p) a -> p t a", p=128))
        Wd_bf = const_pool.tile([128, NEB, A], BF16)
        nc.gpsimd.dma_start(out=Wd_bf, in_=Wd.rearrange("(t p) a -> p t a", p=128))
        vw_bf = const_pool.tile([128, NAT], BF16)
        nc.gpsimd.dma_start(out=vw_bf, in_=vw.rearrange("(t p) -> p t", p=128))
        beT = const_pool.tile([128, NAT], F32)
        nc.sync.dma_start(out=beT, in_=be.rearrange("(t p) -> p t", p=128))
        bdT = const_pool.tile([128, NAT], F32)
        nc.sync.dma_start(out=bdT, in_=bd.rearrange("(t p) -> p t", p=128))
        dh_sb = const_pool.tile([BPC, D], F32)
        nc.sync.dma_start(out=dh_sb, in_=dh)
        ident4 = const_pool.tile([BPC, BPC], F32)
        make_identity(nc, ident4)

        # ---- dec_e^T[a, b] = (dh @ Wd)^T via PE transpose + bf16 matmul ----
        dhT_ps = ps_main.tile([128, NEB, BPC], F32, tag="ee")
        for t in range(NEB):
            nc.tensor.transpose(
                dhT_ps[:, t, :], dh_sb[:, t * 128 : (t + 1) * 128], ident4
            )
        dhT_bf = const_pool.tile([128, NEB, BPC], BF16)
        nc.vector.tensor_copy(dhT_bf, dhT_ps)

        dec_ps = ps_main.tile([128, NAT, BPC], F32, tag="ee")
        for at in range(NAT):
            for t in range(NEB):
                nc.tensor.matmul(
                    dec_ps[:, at, :],
                    Wd_bf[:, t, at * 128 : (at + 1) * 128],
                    dhT_bf[:, t, :],
                    start=(t == 0),
                    stop=(t == NEB - 1),
                )

        # bias[a, at, b] = dec_e^T + be + bd
        comb = const_pool.tile([128, NAT], F32)
        nc.vector.tensor_add(comb, beT, bdT)
        bias_sb = const_pool.tile([128, NAT, BPC], F32)
        for b in range(BPC):
            nc.vector.tensor_add(bias_sb[:, :, b], dec_ps[:, :, b], comb)

        # ---- main per-batch pipeline ----
        nat_tiles = {}
        for b in range(BPC):
            scores_sb = small_pool.tile([1, S], F32, tag="scores_sb")
            for c in range(NCH):
                encT = encT_pool.tile([128, NEB, STPC, 128], BF16, tag="encT")
                for st4 in range(STPC):
                    st = c * STPC + st4
                    nat = nat_pool.tile([128, E], BF16, tag="nat")
                    nat_tiles[(b, st)] = nat
                    nc.gpsimd.dma_start(
                        out=nat, in_=enc[b, st * 128 : (st + 1) * 128, :]
                    )
                    for eb in range(NEB):
                        nc.sync.dma_start_transpose(
                            out=encT[:, eb, st4, :],
                            in_=nat[:, eb * 128 : (eb + 1) * 128],
                        )
                scores_ps = ps_small.tile([1, 512], F32, tag="scores")
                for at in range(NAT):
                    ee_ps = ps_main.tile([128, 512], F32, tag="ee")
                    for eb in range(NEB):
                        nc.tensor.matmul(
                            ee_ps,
                            We_bf[:, eb, at * 128 : (at + 1) * 128],
                            encT[:, eb, :, :],
                            start=(eb == 0),
                            stop=(eb == NEB - 1),
                        )
                    energy = energy_pool.tile([128, 512], BF16, tag="energy")
                    nc.scalar.activation(
                        out=energy,
                        in_=ee_ps,
                        func=ATYPE.Tanh,
                        bias=bias_sb[:, at, b : b + 1],
                        scale=1.0,
                    )
                    nc.tensor.matmul(
                        scores_ps,
                        vw_bf[:, at : at + 1],
                        energy,
                        start=(at == 0),
                        stop=(at == NAT - 1),
                    )
                nc.vector.tensor_copy(scores_sb[:, c * 512 : (c + 1) * 512], scores_ps)

            # softmax over S on one partition
            mx = small_pool.tile([1, 1], F32, tag="mx")
            nc.vector.reduce_max(out=mx, in_=scores_sb, axis=mybir.AxisListType.X)
            nmx = small_pool.tile([1, 1], F32, tag="nmx")
            nc.vector.tensor_scalar_mul(nmx, mx, -1.0)
            probs = small_pool.tile([1, S], F32, tag="probs")
            sume = small_pool.tile([1, 1], F32, tag="sume")
            nc.scalar.activation(
                out=probs,
                in_=scores_sb,
                func=ATYPE.Exp,
                bias=nmx[0:1, 0:1],
                scale=1.0,
                accum_out=sume,
            )
            rs = small_pool.tile([1, 1], F32, tag="rs")
            nc.vector.reciprocal(rs, sume)
            attn_f = small_pool.tile([1, S], F32, tag="attn_f")
            nc.vector.tensor_scalar_mul(attn_f, probs, rs[0:1, 0:1])
            nc.sync.dma_start(out=attn_out[b : b + 1, :], in_=attn_f)

            # spray to column layout via a DRAM bounce (partition redistribution
            # is only expressible with a DRAM endpoint): attnT[p, t] = attn[t*128+p]
            scratch = dram_pool.tile([1, S], F32, tag="scratch")
            nc.sync.dma_start(out=scratch, in_=attn_f)
            attnT = small_pool.tile([128, NST], BF16, tag="attnT")
            nc.gpsimd.dma_start(
                out=attnT, in_=scratch.rearrange("o (t p) -> o p t", p=128)
            )

            # pass 2: context[e] = sum_s attn[s] * enc[s, e]
            ctx_sb = small_pool.tile([1, E], F32, tag="ctx_sb")
            for eh in range(2):
                ctx_ps = ps_small.tile([1, 512], F32, tag="ctx")
                for st in range(NST):
                    nc.tensor.matmul(
                        ctx_ps,
                        attnT[:, st : st + 1],
                        nat_tiles[(b, st)][:, eh * 512 : (eh + 1) * 512],
                        start=(st == 0),
                        stop=(st == NST - 1),
                    )
                nc.vector.tensor_copy(ctx_sb[:, eh * 512 : (eh + 1) * 512], ctx_ps)
            nc.sync.dma_start(out=ctx_out[b : b + 1, :], in_=ctx_sb)


def build_nc():
    nc = bacc.Bacc("TRN2", target_bir_lowering=False, debug=False)
    enc = nc.dram_tensor("enc", [BPC, S, E], F32, kind="ExternalInput").ap()
    dh = nc.dram_tensor("dh", [BPC, D], F32, kind="ExternalInput").ap()
    We = nc.dram_tensor("We", [E, A], F32, kind="ExternalInput").ap()
    be = nc.dram_tensor("be", [A], F32, kind="ExternalInput").ap()
    Wd = nc.dram_tensor("Wd", [D, A], F32, kind="ExternalInput").ap()
    bd = nc.dram_tensor("bd", [A], F32, kind="ExternalInput").ap()
    vw = nc.dram_tensor("vw", [A], F32, kind="ExternalInput").ap()
    ctx_out = nc.dram_tensor("ctx_out", [BPC, E], F32, kind="ExternalOutput").ap()
    attn_out = nc.dram_tensor("attn_out", [BPC, S], F32, kind="ExternalOutput").ap()
    with tile.TileContext(nc) as tc:
        build_kernel(tc, enc, dh, We, be, Wd, bd, vw, ctx_out, attn_out)
    nc.compile()
    return nc


_CACHE = {}


def get_nc():
    if "nc" not in _CACHE:
        _CACHE["nc"] = build_nc()
    return _CACHE["nc"]


def make_in_maps(inputs):
    enc = np.asarray(inputs["encoder_outputs"], dtype=np.float32)
    dh = np.asarray(inputs["decoder_hidden"], dtype=np.float32)
    shared = {
        "We": np.asarray(inputs["We"], dtype=np.float32),
        "be": np.asarray(inputs["be"], dtype=np.float32),
        "Wd": np.asarray(inputs["Wd"], dtype=np.float32),
        "bd": np.asarray(inputs["bd"], dtype=np.float32),
        "vw": np.asarray(inputs["v_w"], dtype=np.float32),
    }
    in_maps = []
    for c in range(NCORES):
        m = dict(shared)
        m["enc"] = np.ascontiguousarray(enc[c * BPC : (c + 1) * BPC])
        m["dh"] = np.ascontiguousarray(dh[c * BPC : (c + 1) * BPC])
        in_maps.append(m)
    return in_maps


def run(inputs, trace=False, **kwargs):
    nc = get_nc()
    res = run_bass_kernel_spmd(
        nc, make_in_maps(inputs), core_ids=list(range(NCORES)), trace=trace, **kwargs
    )
    ctx = np.concatenate([res.results[c]["ctx_out"] for c in range(NCORES)], axis=0)
    attn = np.concatenate([res.results[c]["attn_out"] for c in range(NCORES)], axis=0)
    return res, (ctx, attn)


def kernel(**inputs):
    _, out = run(inputs)
    return out


# revision 11
# speedup vs baseline: 1.7550x; 1.7550x over previous
"""Bahdanau attention kernel for Trainium2 (8 NeuronCores, batch-sharded).

Problem (hardcoded):
  encoder_outputs [32, 2048, 1024] f32, decoder_hidden [32, 1024] f32,
  We [1024, 512], be [512], Wd [1024, 512], bd [512], v_w [512], v_b [] f32.
  Returns (context [32, 1024] f32, attn [32, 2048] f32).

Sharding: data-parallel over batch, 4 batches per core, weights replicated.

Per-core dataflow (batch-pipelined by the Tile scheduler):
  1. SWDGE cast-load enc f32->bf16 into native [128s, 1024e] tiles (kept for
     pass 2), xbar-DMA-transpose 128x128 blocks into encT [128e, s] layout.
  2. PE bf16 matmuls (lhsT=We block, rhs=encT) accumulate enc_e^T [a, s] in
     PSUM over the 8 e-blocks; ACT applies tanh with per-partition bias
     (dec_e + be + bd)[a]; PE contracts with v_w -> scores [1, S].
  3. Row softmax: DVE max, ACT exp (fused sum via accum_out), DVE scale.
  4. One SBUF->SBUF DMA sprays attn into column layout attnT [128s, 16st].
  5. PE pass-2 matmuls (lhsT=attnT col, rhs=native bf16 enc) accumulate
     context [1, 1024] over the 16 s-tiles.
"""

import numpy as np

import concourse.bass as bass
import concourse.mybir as mybir
import concourse.tile as tile
from concourse import bacc
from concourse.bass_utils import run_bass_kernel_spmd
from concourse.masks import make_identity

F32 = mybir.dt.float32
BF16 = mybir.dt.bfloat16

B, S, E, D, A = 32, 2048, 1024, 1024, 512
NCORES = 8
BPC = B // NCORES          # batches per core = 4
NST = S // 128             # s-tiles per batch = 16
NCH = 4                    # chunks per batch (4 s-tiles each)
STPC = NST // NCH          # s-tiles per chunk = 4
NEB = E // 128             # e blocks = 8
NAT = A // 128             # a tiles = 4
ATYPE = mybir.ActivationFunctionType


def build_kernel(tc, enc, dh, We, be, Wd, bd, vw, ctx_out, attn_out):
    nc = tc.nc

    with (
        tc.tile_pool(name="const", bufs=1) as const_pool,
        tc.tile_pool(name="enc_nat", bufs=10) as nat_pool,
        tc.tile_pool(name="encT", bufs=3) as encT_pool,
        tc.tile_pool(name="energy", bufs=6) as energy_pool,
        tc.tile_pool(name="small", bufs=2) as small_pool,
        tc.tile_pool(name="ps_main", bufs=4, space="PSUM") as ps_main,
        tc.tile_pool(name="ps_small", bufs=2, space="PSUM") as ps_small,
        tc.tile_pool(name="dram", bufs=2, space="DRAM") as dram_pool,
    ):
        # ---- replicated weights (a = at*128 + p layout for the a axis) ----
        We_bf = const_pool.tile([128, NEB, A], BF16)
        nc.gpsimd.dma_start(out=We_bf, in_=We.rearrange("(t p) a -> p t a", p=128))
        Wd_bf = const_pool.tile([128, NEB, A], BF16)
        nc.gpsimd.dma_start(out=Wd_bf, in_=Wd.rearrange("(t p) a -> p t a", p=128))
        vw_bf = const_pool.tile([128, NAT], BF16)
        nc.gpsimd.dma_start(out=vw_bf, in_=vw.rearrange("(t p) -> p t", p=128))
        beT = const_pool.tile([128, NAT], F32)
        nc.sync.dma_start(out=beT, in_=be.rearrange("(t p) -> p t", p=128))
        bdT = const_pool.tile([128, NAT], F32)
        nc.sync.dma_start(out=bdT, in_=bd.rearrange("(t p) -> p t", p=128))
        dh_sb = const_pool.tile([BPC, D], F32)
        nc.sync.dma_start(out=dh_sb, in_=dh)
        ident4 = const_pool.tile([BPC, BPC], F32)
        make_identity(nc, ident4)

        # ---- dec_e^T[a, b] = (dh @ Wd)^T via PE transpose + bf16 matmul ----
        dhT_ps = ps_main.tile([128, NEB, BPC], F32, tag="ee")
        for t in range(NEB):
            nc.tensor.transpose(
                dhT_ps[:, t, :], dh_sb[:, t * 128 : (t + 1) * 128], ident4
            )
        dhT_bf = const_pool.tile([128, NEB, BPC], BF16)
        nc.vector.tensor_copy(dhT_bf, dhT_ps)

        dec_ps = ps_main.tile([128, NAT, BPC], F32, tag="ee")
        for at in range(NAT):
            for t in range(NEB):
                nc.tensor.matmul(
                    dec_ps[:, at, :],
                    Wd_bf[:, t, at * 128 : (at + 1) * 128],
                    dhT_bf[:, t, :],
                    start=(t == 0),
                    stop=(t == NEB - 1),
                )

        # bias[a, at, b] = dec_e^T + be + bd
        comb = const_pool.tile([128, NAT], F32)
        nc.vector.tensor_add(comb, beT, bdT)
        bias_sb = const_pool.tile([128, NAT, BPC], F32)
        for b in range(BPC):
            nc.vector.tensor_add(bias_sb[:, :, b], dec_ps[:, :, b], comb)

        # ---- main per-batch pipeline ----
        nat_tiles = {}
        attnT_tiles = {}

        def pass1_batch(b):
            scores_sb = small_pool.tile([1, S], F32, tag="scores_sb")
            for c in range(NCH):
                # one 2 MiB SWDGE cast-load per chunk: [512s, 1024e] f32 -> bf16
                nat = nat_pool.tile([128, STPC, E], BF16, tag="nat")
                nat_tiles[(b, c)] = nat
                nc.gpsimd.dma_start(
                    out=nat,
                    in_=enc[b, c * 512 : (c + 1) * 512, :].rearrange(
                        "(st p) e -> p st e", p=128
                    ),
                )
                # one xbar transpose per chunk: block-transposes each 128-col
                # group of in; st-major layout makes the out AP merge to 3D.
                encT = encT_pool.tile([128, STPC, NEB, 128], BF16, tag="encT")
                nc.sync.dma_start_transpose(out=encT, in_=nat)
                scores_ps = ps_small.tile([1, 512], F32, tag="scores")
                for at in range(NAT):
                    ee_ps = ps_main.tile([128, 512], F32, tag="ee")
                    for eb in range(NEB):
                        nc.tensor.matmul(
                            ee_ps,
                            We_bf[:, eb, at * 128 : (at + 1) * 128],
                            encT[:, :, eb, :],
                            start=(eb == 0),
                            stop=(eb == NEB - 1),
                        )
                    energy = energy_pool.tile([128, 512], BF16, tag="energy")
                    nc.scalar.activation(
                        out=energy,
                        in_=ee_ps,
                        func=ATYPE.Tanh,
                        bias=bias_sb[:, at, b : b + 1],
                        scale=1.0,
                    )
                    nc.tensor.matmul(
                        scores_ps,
                        vw_bf[:, at : at + 1],
                        energy,
                        start=(at == 0),
                        stop=(at == NAT - 1),
                    )
                nc.vector.tensor_copy(scores_sb[:, c * 512 : (c + 1) * 512], scores_ps)

            # softmax over S on one partition
            mx = small_pool.tile([1, 1], F32, tag="mx")
            nc.vector.reduce_max(out=mx, in_=scores_sb, axis=mybir.AxisListType.X)
            nmx = small_pool.tile([1, 1], F32, tag="nmx")
            nc.vector.tensor_scalar_mul(nmx, mx, -1.0)
            probs = small_pool.tile([1, S], F32, tag="probs")
            sume = small_pool.tile([1, 1], F32, tag="sume")
            nc.scalar.activation(
                out=probs,
                in_=scores_sb,
                func=ATYPE.Exp,
                bias=nmx[0:1, 0:1],
                scale=1.0,
                accum_out=sume,
            )
            rs = small_pool.tile([1, 1], F32, tag="rs")
            nc.vector.reciprocal(rs, sume)
            attn_f = small_pool.tile([1, S], F32, tag="attn_f")
            nc.vector.tensor_scalar_mul(attn_f, probs, rs[0:1, 0:1])
            nc.sync.dma_start(out=attn_out[b : b + 1, :], in_=attn_f)

            # spray to column layout via a DRAM bounce (partition redistribution
            # is only expressible with a DRAM endpoint): attnT[p, t] = attn[t*128+p]
            scratch = dram_pool.tile([1, S], F32, tag="scratch")
            nc.sync.dma_start(out=scratch, in_=attn_f)
            attnT = small_pool.tile([128, NST], BF16, tag="attnT")
            nc.gpsimd.dma_start(
                out=attnT, in_=scratch.rearrange("o (t p) -> o p t", p=128)
            )
            attnT_tiles[b] = attnT

        def pass2_batch(b):
            # pass 2: context[e] = sum_s attn[s] * enc[s, e]
            attnT = attnT_tiles[b]
            ctx_sb = small_pool.tile([1, E], F32, tag="ctx_sb")
            for eh in range(2):
                ctx_ps = ps_small.tile([1, 512], F32, tag="ctx")
                for st in range(NST):
                    nc.tensor.matmul(
                        ctx_ps,
                        attnT[:, st : st + 1],
                        nat_tiles[(b, st // STPC)][
                            :, st % STPC, eh * 512 : (eh + 1) * 512
                        ],
                        start=(st == 0),
                        stop=(st == NST - 1),
                    )
                nc.vector.tensor_copy(ctx_sb[:, eh * 512 : (eh + 1) * 512], ctx_ps)
            nc.sync.dma_start(out=ctx_out[b : b + 1, :], in_=ctx_sb)

        # software pipeline: emit pass2(b) after pass1(b+1) so the PE stream
        # never waits on batch b's softmax/spray chain.
        pass1_batch(0)
        for b in range(1, BPC):
            pass1_batch(b)
            pass2_batch(b - 1)
        pass2_batch(BPC - 1)


def build_nc():
    nc = bacc.Bacc("TRN2", target_bir_lowering=False, debug=False)
    enc = nc.dram_tensor("enc", [BPC, S, E], F32, kind="ExternalInput").ap()
    dh = nc.dram_tensor("dh", [BPC, D], F32, kind="ExternalInput").ap()
    We = nc.dram_tensor("We", [E, A], F32, kind="ExternalInput").ap()
    be = nc.dram_tensor("be", [A], F32, kind="ExternalInput").ap()
    Wd = nc.dram_tensor("Wd", [D, A], F32, kind="ExternalInput").ap()
    bd = nc.dram_tensor("bd", [A], F32, kind="ExternalInput").ap()
    vw = nc.dram_tensor("vw", [A], F32, kind="ExternalInput").ap()
    ctx_out = nc.dram_tensor("ctx_out", [BPC, E], F32, kind="ExternalOutput").ap()
    attn_out = nc.dram_tensor("attn_out", [BPC, S], F32, kind="ExternalOutput").ap()
    with tile.TileContext(nc) as tc:
        build_kernel(tc, enc, dh, We, be, Wd, bd, vw, ctx_out, attn_out)
    nc.compile()
    return nc


_CACHE = {}


def get_nc():
    if "nc" not in _CACHE:
        _CACHE["nc"] = build_nc()
    return _CACHE["nc"]


def make_in_maps(inputs):
    enc = np.asarray(inputs["encoder_outputs"], dtype=np.float32)
    dh = np.asarray(inputs["decoder_hidden"], dtype=np.float32)
    shared = {
        "We": np.asarray(inputs["We"], dtype=np.float32),
        "be": np.asarray(inputs["be"], dtype=np.float32),
        "Wd": np.asarray(inputs["Wd"], dtype=np.float32),
        "bd": np.asarray(inputs["bd"], dtype=np.float32),
        "vw": np.asarray(inputs["v_w"], dtype=np.float32),
    }
    in_maps = []
    for c in range(NCORES):
        m = dict(shared)
        m["enc"] = np.ascontiguousarray(enc[c * BPC : (c + 1) * BPC])
        m["dh"] = np.ascontiguousarray(dh[c * BPC : (c + 1) * BPC])
        in_maps.append(m)
    return in_maps


def run(inputs, trace=False, **kwargs):
    nc = get_nc()
    res = run_bass_kernel_spmd(
        nc, make_in_maps(inputs), core_ids=list(range(NCORES)), trace=trace, **kwargs
    )
    ctx = np.concatenate([res.results[c]["ctx_out"] for c in range(NCORES)], axis=0)
    attn = np.concatenate([res.results[c]["attn_out"] for c in range(NCORES)], axis=0)
    return res, (ctx, attn)


def kernel(**inputs):
    _, out = run(inputs)
    return out


# revision 12
# speedup vs baseline: 2.0469x; 1.1664x over previous
"""Bahdanau attention kernel for Trainium2 (8 NeuronCores, batch-sharded).

Problem (hardcoded):
  encoder_outputs [32, 2048, 1024] f32, decoder_hidden [32, 1024] f32,
  We [1024, 512], be [512], Wd [1024, 512], bd [512], v_w [512], v_b [] f32.
  Returns (context [32, 1024] f32, attn [32, 2048] f32).

Sharding: data-parallel over batch, 4 batches per core, weights replicated.

Per-core dataflow (batch-pipelined by the Tile scheduler):
  1. SWDGE cast-load enc f32->bf16 into native [128s, 1024e] tiles (kept for
     pass 2), xbar-DMA-transpose 128x128 blocks into encT [128e, s] layout.
  2. PE bf16 matmuls (lhsT=We block, rhs=encT) accumulate enc_e^T [a, s] in
     PSUM over the 8 e-blocks; ACT applies tanh with per-partition bias
     (dec_e + be + bd)[a]; PE contracts with v_w -> scores [1, S].
  3. Row softmax: DVE max, ACT exp (fused sum via accum_out), DVE scale.
  4. One SBUF->SBUF DMA sprays attn into column layout attnT [128s, 16st].
  5. PE pass-2 matmuls (lhsT=attnT col, rhs=native bf16 enc) accumulate
     context [1, 1024] over the 16 s-tiles.
"""

import numpy as np

import concourse.bass as bass
import concourse.mybir as mybir
import concourse.tile as tile
from concourse import bacc
from concourse.bass_utils import run_bass_kernel_spmd
from concourse.masks import make_identity

F32 = mybir.dt.float32
BF16 = mybir.dt.bfloat16

B, S, E, D, A = 32, 2048, 1024, 1024, 512
NCORES = 8
BPC = B // NCORES          # batches per core = 4
NST = S // 128             # s-tiles per batch = 16
NCH = 4                    # chunks per batch (4 s-tiles each)
STPC = NST // NCH          # s-tiles per chunk = 4
NEB = E // 128             # e blocks = 8
NAT = A // 128             # a tiles = 4
ATYPE = mybir.ActivationFunctionType


def build_kernel(tc, enc, dh, We, be, Wd, bd, vw, ctx_out, attn_out):
    nc = tc.nc

    with (
        tc.tile_pool(name="const", bufs=1) as const_pool,
        tc.tile_pool(name="enc_nat", bufs=10) as nat_pool,
        tc.tile_pool(name="encT", bufs=3) as encT_pool,
        tc.tile_pool(name="energy", bufs=6) as energy_pool,
        tc.tile_pool(name="small", bufs=2) as small_pool,
        tc.tile_pool(name="ps_main", bufs=4, space="PSUM") as ps_main,
        tc.tile_pool(name="ps_small", bufs=2, space="PSUM") as ps_small,
        tc.tile_pool(name="dram", bufs=2, space="DRAM") as dram_pool,
    ):
        # ---- replicated weights (a = at*128 + p layout for the a axis) ----
        We_bf = const_pool.tile([128, NEB, A], BF16)
        nc.gpsimd.dma_start(out=We_bf, in_=We.rearrange("(t p) a -> p t a", p=128))
        Wd_bf = const_pool.tile([128, NEB, A], BF16)
        nc.gpsimd.dma_start(out=Wd_bf, in_=Wd.rearrange("(t p) a -> p t a", p=128))
        vw_bf = const_pool.tile([128, NAT], BF16)
        nc.gpsimd.dma_start(out=vw_bf, in_=vw.rearrange("(t p) -> p t", p=128))
        beT = const_pool.tile([128, NAT], F32)
        nc.sync.dma_start(out=beT, in_=be.rearrange("(t p) -> p t", p=128))
        bdT = const_pool.tile([128, NAT], F32)
        nc.sync.dma_start(out=bdT, in_=bd.rearrange("(t p) -> p t", p=128))
        dh_sb = const_pool.tile([BPC, D], F32)
        nc.sync.dma_start(out=dh_sb, in_=dh)
        ident4 = const_pool.tile([BPC, BPC], F32)
        make_identity(nc, ident4)

        # ---- dec_e^T[a, b] = (dh @ Wd)^T via PE transpose + bf16 matmul ----
        dhT_ps = ps_main.tile([128, NEB, BPC], F32, tag="ee")
        for t in range(NEB):
            nc.tensor.transpose(
                dhT_ps[:, t, :], dh_sb[:, t * 128 : (t + 1) * 128], ident4
            )
        dhT_bf = const_pool.tile([128, NEB, BPC], BF16)
        nc.vector.tensor_copy(dhT_bf, dhT_ps)

        dec_ps = ps_main.tile([128, NAT, BPC], F32, tag="ee")
        for at in range(NAT):
            for t in range(NEB):
                nc.tensor.matmul(
                    dec_ps[:, at, :],
                    Wd_bf[:, t, at * 128 : (at + 1) * 128],
                    dhT_bf[:, t, :],
                    start=(t == 0),
                    stop=(t == NEB - 1),
                )

        # bias[a, at, b] = dec_e^T + be + bd
        comb = const_pool.tile([128, NAT], F32)
        nc.vector.tensor_add(comb, beT, bdT)
        bias_sb = const_pool.tile([128, NAT, BPC], F32)
        for b in range(BPC):
            nc.vector.tensor_add(bias_sb[:, :, b], dec_ps[:, :, b], comb)

        # ---- main per-batch pipeline ----
        nat_tiles = {}
        attnT_tiles = {}

        def pass1_batch(b):
            scores_sb = small_pool.tile([1, S], F32, tag="scores_sb")
            for c in range(NCH):
                # one 2 MiB SWDGE cast-load per chunk: [512s, 1024e] f32 -> bf16
                nat = nat_pool.tile([128, STPC, E], BF16, tag="nat")
                nat_tiles[(b, c)] = nat
                nc.gpsimd.dma_start(
                    out=nat,
                    in_=enc[b, c * 512 : (c + 1) * 512, :].rearrange(
                        "(st p) e -> p st e", p=128
                    ),
                )
                # one xbar transpose per chunk: block-transposes each 128-col
                # group of in; st-major layout makes the out AP merge to 3D.
                encT = encT_pool.tile([128, STPC, NEB, 128], BF16, tag="encT")
                nc.sync.dma_start_transpose(out=encT, in_=nat)
                scores_ps = ps_small.tile([1, 512], F32, tag="scores")
                for at in range(NAT):
                    ee_ps = ps_main.tile([128, 512], F32, tag="ee")
                    for eb in range(NEB):
                        nc.tensor.matmul(
                            ee_ps,
                            We_bf[:, eb, at * 128 : (at + 1) * 128],
                            encT[:, :, eb, :],
                            start=(eb == 0),
                            stop=(eb == NEB - 1),
                        )
                    energy = energy_pool.tile([128, 512], BF16, tag="energy")
                    nc.scalar.activation(
                        out=energy,
                        in_=ee_ps,
                        func=ATYPE.Tanh,
                        bias=bias_sb[:, at, b : b + 1],
                        scale=1.0,
                    )
                    nc.tensor.matmul(
                        scores_ps,
                        vw_bf[:, at : at + 1],
                        energy,
                        start=(at == 0),
                        stop=(at == NAT - 1),
                    )
                nc.vector.tensor_copy(scores_sb[:, c * 512 : (c + 1) * 512], scores_ps)

            # softmax over S on one partition
            mx = small_pool.tile([1, 1], F32, tag="mx")
            nc.vector.reduce_max(out=mx, in_=scores_sb, axis=mybir.AxisListType.X)
            nmx = small_pool.tile([1, 1], F32, tag="nmx")
            nc.vector.tensor_scalar_mul(nmx, mx, -1.0)
            probs = small_pool.tile([1, S], F32, tag="probs")
            sume = small_pool.tile([1, 1], F32, tag="sume")
            nc.scalar.activation(
                out=probs,
                in_=scores_sb,
                func=ATYPE.Exp,
                bias=nmx[0:1, 0:1],
                scale=1.0,
                accum_out=sume,
            )
            rs = small_pool.tile([1, 1], F32, tag="rs")
            nc.vector.reciprocal(rs, sume)
            attn_f = small_pool.tile([1, S], F32, tag="attn_f")
            nc.vector.tensor_scalar_mul(attn_f, probs, rs[0:1, 0:1])
            nc.sync.dma_start(out=attn_out[b : b + 1, :], in_=attn_f)

            # spray to column layout via a DRAM bounce (partition redistribution
            # is only expressible with a DRAM endpoint): attnT[p, t] = attn[t*128+p].
            # Both hops ride the two HWDGE rings (SP write, ACT read) so the
            # SWDGE queue stays free for enc chunk loads (no head-of-line block).
            attn_bf = small_pool.tile([1, S], BF16, tag="attn_bf")
            nc.vector.tensor_copy(attn_bf, attn_f)
            scratch = dram_pool.tile([1, S], BF16, tag="scratch")
            nc.sync.dma_start(out=scratch, in_=attn_bf)
            attnT = small_pool.tile([128, NST], BF16, tag="attnT")
            nc.scalar.dma_start(
                out=attnT, in_=scratch.rearrange("o (t p) -> o p t", p=128)
            )
            attnT_tiles[b] = attnT

        def pass2_batch(b):
            # pass 2: context[e] = sum_s attn[s] * enc[s, e]
            attnT = attnT_tiles[b]
            ctx_sb = small_pool.tile([1, E], F32, tag="ctx_sb")
            for eh in range(2):
                ctx_ps = ps_small.tile([1, 512], F32, tag="ctx")
                for st in range(NST):
                    nc.tensor.matmul(
                        ctx_ps,
                        attnT[:, st : st + 1],
                        nat_tiles[(b, st // STPC)][
                            :, st % STPC, eh * 512 : (eh + 1) * 512
                        ],
                        start=(st == 0),
                        stop=(st == NST - 1),
                    )
                nc.vector.tensor_copy(ctx_sb[:, eh * 512 : (eh + 1) * 512], ctx_ps)
            nc.sync.dma_start(out=ctx_out[b : b + 1, :], in_=ctx_sb)

        # software pipeline: emit pass2(b) after pass1(b+1) so the PE stream
        # never waits on batch b's softmax/spray chain.
        pass1_batch(0)
        for b in range(1, BPC):
            pass1_batch(b)
            pass2_batch(b - 1)
        pass2_batch(BPC - 1)


def build_nc():
    nc = bacc.Bacc("TRN2", target_bir_lowering=False, debug=False)
    enc = nc.dram_tensor("enc", [BPC, S, E], F32, kind="ExternalInput").ap()
    dh = nc.dram_tensor("dh", [BPC, D], F32, kind="ExternalInput").ap()
    We = nc.dram_tensor("We", [E, A], F32, kind="ExternalInput").ap()
    be = nc.dram_tensor("be", [A], F32, kind="ExternalInput").ap()
    Wd = nc.dram_tensor("Wd", [D, A], F32, kind="ExternalInput").ap()
    bd = nc.dram_tensor("bd", [A], F32, kind="ExternalInput").ap()
    vw = nc.dram_tensor("vw", [A], F32, kind="ExternalInput").ap()
    ctx_out = nc.dram_tensor("ctx_out", [BPC, E], F32, kind="ExternalOutput").ap()
    attn_out = nc.dram_tensor("attn_out", [BPC, S], F32, kind="ExternalOutput").ap()
    with tile.TileContext(nc) as tc:
        build_kernel(tc, enc, dh, We, be, Wd, bd, vw, ctx_out, attn_out)
    nc.compile()
    return nc


_CACHE = {}


def get_nc():
    if "nc" not in _CACHE:
        _CACHE["nc"] = build_nc()
    return _CACHE["nc"]


def make_in_maps(inputs):
    enc = np.asarray(inputs["encoder_outputs"], dtype=np.float32)
    dh = np.asarray(inputs["decoder_hidden"], dtype=np.float32)
    shared = {
        "We": np.asarray(inputs["We"], dtype=np.float32),
        "be": np.asarray(inputs["be"], dtype=np.float32),
        "Wd": np.asarray(inputs["Wd"], dtype=np.float32),
        "bd": np.asarray(inputs["bd"], dtype=np.float32),
        "vw": np.asarray(inputs["v_w"], dtype=np.float32),
    }
    in_maps = []
    for c in range(NCORES):
        m = dict(shared)
        m["enc"] = np.ascontiguousarray(enc[c * BPC : (c + 1) * BPC])
        m["dh"] = np.ascontiguousarray(dh[c * BPC : (c + 1) * BPC])
        in_maps.append(m)
    return in_maps


def run(inputs, trace=False, **kwargs):
    nc = get_nc()
    res = run_bass_kernel_spmd(
        nc, make_in_maps(inputs), core_ids=list(range(NCORES)), trace=trace, **kwargs
    )
    ctx = np.concatenate([res.results[c]["ctx_out"] for c in range(NCORES)], axis=0)
    attn = np.concatenate([res.results[c]["attn_out"] for c in range(NCORES)], axis=0)
    return res, (ctx, attn)


def kernel(**inputs):
    _, out = run(inputs)
    return out


# revision 18
# speedup vs baseline: 18.6063x; 9.0899x over previous
"""Bahdanau attention kernel for Trainium2 (8 NeuronCores, batch-sharded).

Problem (hardcoded):
  encoder_outputs [32, 2048, 1024] f32, decoder_hidden [32, 1024] f32,
  We [1024, 512], be [512], Wd [1024, 512], bd [512], v_w [512], v_b [] f32.
  Returns (context [32, 1024] f32, attn [32, 2048] f32).

Sharding: data-parallel over batch, 4 batches per core, weights replicated.

Per-core dataflow (batch-pipelined by the Tile scheduler):
  1. SWDGE cast-load enc f32->bf16 into native [128s, 1024e] tiles (kept for
     pass 2), xbar-DMA-transpose 128x128 blocks into encT [128e, s] layout.
  2. PE bf16 matmuls (lhsT=We block, rhs=encT) accumulate enc_e^T [a, s] in
     PSUM over the 8 e-blocks; ACT applies tanh with per-partition bias
     (dec_e + be + bd)[a]; PE contracts with v_w -> scores [1, S].
  3. Row softmax: DVE max, ACT exp (fused sum via accum_out), DVE scale.
  4. One SBUF->SBUF DMA sprays attn into column layout attnT [128s, 16st].
  5. PE pass-2 matmuls (lhsT=attnT col, rhs=native bf16 enc) accumulate
     context [1, 1024] over the 16 s-tiles.
"""

import numpy as np

import concourse.bass as bass
import concourse.mybir as mybir
import concourse.tile as tile
from concourse import bacc
from concourse.bass_utils import run_bass_kernel_spmd
from concourse.masks import make_identity

F32 = mybir.dt.float32
BF16 = mybir.dt.bfloat16

B, S, E, D, A = 32, 2048, 1024, 1024, 512
NCORES = 8
BPC = B // NCORES          # batches per core = 4
NST = S // 128             # s-tiles per batch = 16
NCH = 4                    # chunks per batch (4 s-tiles each)
STPC = NST // NCH          # s-tiles per chunk = 4
NEB = E // 128             # e blocks = 8
NAT = A // 128             # a tiles = 4
ATYPE = mybir.ActivationFunctionType


def build_kernel(tc, enc, dh, We, be, Wd, bd, vw, ctx_out, attn_out, iters=1):
    nc = tc.nc

    with (
        tc.tile_pool(name="const", bufs=1) as const_pool,
        tc.tile_pool(name="enc_nat", bufs=10) as nat_pool,
        tc.tile_pool(name="encT", bufs=3) as encT_pool,
        tc.tile_pool(name="energy", bufs=6) as energy_pool,
        tc.tile_pool(name="small", bufs=2) as small_pool,
        tc.tile_pool(name="ps_main", bufs=4, space="PSUM") as ps_main,
        tc.tile_pool(name="ps_small", bufs=2, space="PSUM") as ps_small,
        tc.tile_pool(name="dram", bufs=2, space="DRAM") as dram_pool,
    ):
        nat_tiles = {}
        encT_tiles = {}

        def load_chunk(b, c):
            # one 2 MiB SWDGE cast-load per chunk: [512s, 1024e] f32 -> bf16,
            # then one xbar transpose (block-transposes each 128-col group;
            # st-major layout makes the out AP merge to 3D).
            nat = nat_pool.tile([128, STPC, E], BF16, tag="nat")
            nat_tiles[(b, c)] = nat
            nc.gpsimd.dma_start(
                out=nat,
                in_=enc[b, c * 512 : (c + 1) * 512, :].rearrange(
                    "(st p) e -> p st e", p=128
                ),
            )
            encT = encT_pool.tile([128, STPC, NEB, 128], BF16, tag="encT")
            encT_tiles[(b, c)] = encT
            nc.sync.dma_start_transpose(out=encT, in_=nat)

        # ---- replicated weights (a = at*128 + p layout for the a axis) ----
        We_bf = const_pool.tile([128, NEB, A], BF16)
        nc.gpsimd.dma_start(out=We_bf, in_=We.rearrange("(t p) a -> p t a", p=128))
        Wd_bf = const_pool.tile([128, NEB, A], BF16)
        nc.gpsimd.dma_start(out=Wd_bf, in_=Wd.rearrange("(t p) a -> p t a", p=128))
        vw_bf = const_pool.tile([128, NAT], BF16)
        nc.gpsimd.dma_start(out=vw_bf, in_=vw.rearrange("(t p) -> p t", p=128))
        beT = const_pool.tile([128, NAT], F32)
        nc.sync.dma_start(out=beT, in_=be.rearrange("(t p) -> p t", p=128))
        bdT = const_pool.tile([128, NAT], F32)
        nc.sync.dma_start(out=bdT, in_=bd.rearrange("(t p) -> p t", p=128))
        dh_sb = const_pool.tile([BPC, D], F32)
        nc.sync.dma_start(out=dh_sb, in_=dh)
        ident4 = const_pool.tile([BPC, BPC], F32)
        make_identity(nc, ident4)

        # ---- dec_e^T[a, b] = (dh @ Wd)^T via PE transpose + bf16 matmul ----
        dhT_ps = ps_main.tile([128, NEB, BPC], F32, tag="ee")
        for t in range(NEB):
            nc.tensor.transpose(
                dhT_ps[:, t, :], dh_sb[:, t * 128 : (t + 1) * 128], ident4
            )
        dhT_bf = const_pool.tile([128, NEB, BPC], BF16)
        nc.vector.tensor_copy(dhT_bf, dhT_ps)

        dec_ps = ps_main.tile([128, NAT, BPC], F32, tag="ee")
        for at in range(NAT):
            for t in range(NEB):
                nc.tensor.matmul(
                    dec_ps[:, at, :],
                    Wd_bf[:, t, at * 128 : (at + 1) * 128],
                    dhT_bf[:, t, :],
                    start=(t == 0),
                    stop=(t == NEB - 1),
                )

        # bias[a, at, b] = dec_e^T + be + bd
        comb = const_pool.tile([128, NAT], F32)
        nc.vector.tensor_add(comb, beT, bdT)
        bias_sb = const_pool.tile([128, NAT, BPC], F32)
        for b in range(BPC):
            nc.vector.tensor_add(bias_sb[:, :, b], dec_ps[:, :, b], comb)

        # ---- main per-batch pipeline ----
        attnT_tiles = {}
        rs_tiles = {}

        def pass1_batch(b):
            scores_sb = small_pool.tile([1, S], F32, tag="scores_sb")
            cmax = small_pool.tile([1, NCH], F32, tag="cmax")
            for c in range(NCH):
                if (b, c) not in nat_tiles:
                    load_chunk(b, c)
                encT = encT_tiles[(b, c)]
                scores_ps = ps_small.tile([1, 512], F32, tag="scores")
                for at in range(NAT):
                    ee_ps = ps_main.tile([128, 512], F32, tag="ee")
                    for eb in range(NEB):
                        nc.tensor.matmul(
                            ee_ps,
                            We_bf[:, eb, at * 128 : (at + 1) * 128],
                            encT[:, :, eb, :],
                            start=(eb == 0),
                            stop=(eb == NEB - 1),
                        )
                    energy = energy_pool.tile([128, 512], BF16, tag="energy")
                    nc.scalar.activation(
                        out=energy,
                        in_=ee_ps,
                        func=ATYPE.Tanh,
                        bias=bias_sb[:, at, b : b + 1],
                        scale=1.0,
                    )
                    nc.tensor.matmul(
                        scores_ps,
                        vw_bf[:, at : at + 1],
                        energy,
                        start=(at == 0),
                        stop=(at == NAT - 1),
                    )
                nc.vector.tensor_copy(scores_sb[:, c * 512 : (c + 1) * 512], scores_ps)
                # per-chunk max, off the critical softmax chain
                nc.vector.reduce_max(
                    out=cmax[0:1, c : c + 1], in_=scores_ps, axis=mybir.AxisListType.X
                )

            # softmax over S on one partition; the spray carries unnormalized
            # probs (1/sum is folded into context after pass 2).
            mx = small_pool.tile([1, 1], F32, tag="mx")
            nc.vector.reduce_max(out=mx, in_=cmax, axis=mybir.AxisListType.X)
            nmx = small_pool.tile([1, 1], F32, tag="nmx")
            nc.vector.tensor_scalar_mul(nmx, mx, -1.0)
            probs = small_pool.tile([1, S], F32, tag="probs")
            sume = small_pool.tile([1, 1], F32, tag="sume")
            nc.scalar.activation(
                out=probs,
                in_=scores_sb,
                func=ATYPE.Exp,
                bias=nmx[0:1, 0:1],
                scale=1.0,
                accum_out=sume,
            )
            rs = small_pool.tile([1, 1], F32, tag="rs")
            nc.vector.reciprocal(rs, sume)
            rs_tiles[b] = rs

            # spray to column layout via a DRAM bounce (partition redistribution
            # is only expressible with a DRAM endpoint): probsT[p, t] = probs[t*128+p].
            # Both hops ride the two HWDGE rings so the SWDGE queue stays free
            # for enc chunk loads (no head-of-line block).
            probs_bf = small_pool.tile([1, S], BF16, tag="probs_bf")
            nc.vector.tensor_copy(probs_bf, probs)
            scratch = dram_pool.tile([1, S], BF16, tag="scratch")
            nc.scalar.dma_start(out=scratch, in_=probs_bf)
            attnT = small_pool.tile([128, NST], BF16, tag="attnT")
            nc.scalar.dma_start(
                out=attnT, in_=scratch.rearrange("o (t p) -> o p t", p=128)
            )
            attnT_tiles[b] = attnT

            # normalized attn output (off the pass-2 critical path)
            attn_f = small_pool.tile([1, S], F32, tag="attn_f")
            nc.vector.tensor_scalar_mul(attn_f, probs, rs[0:1, 0:1])
            nc.scalar.dma_start(out=attn_out[b : b + 1, :], in_=attn_f)

        def pass2_batch(b):
            # pass 2: context[e] = sum_s attn[s] * enc[s, e]
            attnT = attnT_tiles[b]
            ctx_sb = small_pool.tile([1, E], F32, tag="ctx_sb")
            for eh in range(2):
                ctx_ps = ps_small.tile([1, 512], F32, tag="ctx")
                for st in range(NST):
                    nc.tensor.matmul(
                        ctx_ps,
                        attnT[:, st : st + 1],
                        nat_tiles[(b, st // STPC)][
                            :, st % STPC, eh * 512 : (eh + 1) * 512
                        ],
                        start=(st == 0),
                        stop=(st == NST - 1),
                    )
                nc.vector.tensor_scalar_mul(
                    ctx_sb[:, eh * 512 : (eh + 1) * 512],
                    ctx_ps,
                    rs_tiles[b][0:1, 0:1],
                )
            nc.scalar.dma_start(out=ctx_out[b : b + 1, :], in_=ctx_sb)

        # software pipeline: emit pass2(b) after pass1(b+1) so the PE stream
        # never waits on batch b's softmax/spray chain.
        for _ in range(iters):
            nat_tiles.clear()
            encT_tiles.clear()
            attnT_tiles.clear()
            rs_tiles.clear()
            pass1_batch(0)
            for b in range(1, BPC):
                pass1_batch(b)
                pass2_batch(b - 1)
            pass2_batch(BPC - 1)


def build_nc(iters=1):
    nc = bacc.Bacc("TRN2", target_bir_lowering=False, debug=False)
    enc = nc.dram_tensor("enc", [BPC, S, E], F32, kind="ExternalInput").ap()
    dh = nc.dram_tensor("dh", [BPC, D], F32, kind="ExternalInput").ap()
    We = nc.dram_tensor("We", [E, A], F32, kind="ExternalInput").ap()
    be = nc.dram_tensor("be", [A], F32, kind="ExternalInput").ap()
    Wd = nc.dram_tensor("Wd", [D, A], F32, kind="ExternalInput").ap()
    bd = nc.dram_tensor("bd", [A], F32, kind="ExternalInput").ap()
    vw = nc.dram_tensor("vw", [A], F32, kind="ExternalInput").ap()
    ctx_out = nc.dram_tensor("ctx_out", [BPC, E], F32, kind="ExternalOutput").ap()
    attn_out = nc.dram_tensor("attn_out", [BPC, S], F32, kind="ExternalOutput").ap()
    with tile.TileContext(nc) as tc:
        build_kernel(tc, enc, dh, We, be, Wd, bd, vw, ctx_out, attn_out, iters=iters)
    nc.compile()
    return nc


_CACHE = {}


def get_nc(iters=1):
    key = ("nc", iters)
    if key not in _CACHE:
        _CACHE[key] = build_nc(iters=iters)
    return _CACHE[key]


def make_in_maps(inputs):
    enc = np.asarray(inputs["encoder_outputs"], dtype=np.float32)
    dh = np.asarray(inputs["decoder_hidden"], dtype=np.float32)
    shared = {
        "We": np.asarray(inputs["We"], dtype=np.float32),
        "be": np.asarray(inputs["be"], dtype=np.float32),
        "Wd": np.asarray(inputs["Wd"], dtype=np.float32),
        "bd": np.asarray(inputs["bd"], dtype=np.float32),
        "vw": np.asarray(inputs["v_w"], dtype=np.float32),
    }
    in_maps = []
    for c in range(NCORES):
        m = dict(shared)
        m["enc"] = np.ascontiguousarray(enc[c * BPC : (c + 1) * BPC])
        m["dh"] = np.ascontiguousarray(dh[c * BPC : (c + 1) * BPC])
        in_maps.append(m)
    return in_maps


def run(inputs, trace=False, **kwargs):
    nc = get_nc()
    res = run_bass_kernel_spmd(
        nc, make_in_maps(inputs), core_ids=list(range(NCORES)), trace=trace, **kwargs
    )
    ctx = np.concatenate([res.results[c]["ctx_out"] for c in range(NCORES)], axis=0)
    attn = np.concatenate([res.results[c]["attn_out"] for c in range(NCORES)], axis=0)
    return res, (ctx, attn)


def kernel(**inputs):
    _, out = run(inputs)
    return out
